# revision 14
# baseline (speedup 1.0000x reference)
"""HardBinaryVote Trainium2 kernel.

out[s] = (sum_m w[m]*votes[m,s] > sum_m w[m]/2)  as int32, votes in {0,1}.

Strategy (8 NeuronCores, sample-sharded):
  - Each core gets a [63, 250000] shard of votes, folded host-side into
    [126, 125000] (two fold-halves stacked on the partition axis), padded to
    126976 columns (248 chunks of 512), encoded as fp8 e4m3 {0.0, 1.0}
    (1 byte/vote -> plain HWDGE DMA at the ~360 GB/s HBM-per-core limit).
    Input is pre-tiled host-side into 31 contiguous 516 KB ranges
    ([31, 126, 4096]) so every range DMA is a single linear DRAM read;
    ranges alternate between the sync and gpsimd queues.
  - Weights quantized once to fp16 (exact-decision mismatch count vs the
    fp32 reference measured at 135/2M, rel_err 0.012 < 2e-2), laid out
    [126, 2] block-diagonal over the two fold-halves.
  - Single matmul pass, mixed dtype (e4m3 moving x fp16 stationary), with
    4-way PE column tiling: chunks round-robin tile_position (0, 32j), so
    4 matmuls stream concurrently (~61 ns per 512-col matmul when warm).
    A short dummy-matmul burst at kernel start warms the PE clock gate
    (HAM) while the first input ranges are still in flight.
  - Per PSUM bank (4 chunks), threshold alternates between DVE
    tensor_scalar(is_gt, T) -> {0,1} and ACT Sign(y - T) -> {-1,0,1};
    host maps >0 to 1. int8 outputs, batched sparse-partition DMAs.
"""

import sys

import numpy as np

sys.path.insert(0, "/opt/trn_rl_repo")

import ml_dtypes  # noqa: E402

from concourse import bacc, bass_utils, mybir, tile  # noqa: E402

N_MODELS = 63
N_SAMPLES = 2_000_000
N_CORES = 8
S_CORE = N_SAMPLES // N_CORES  # 250000 samples per core
H = S_CORE // 2  # 125000 real columns per core (2 samples each)
KP = 2 * N_MODELS  # 126 contraction rows

C = 512  # matmul free dim / PSUM bank
NCH = 248  # chunks per core (padded)
W = NCH * C  # 126976 padded columns
HGRP = NCH // 8  # 31 psum banks per output half (bank = 4 chunks)
OW = HGRP * C  # 15872 output columns per half per (j, fold) row

RW = 8  # chunks per input DMA range
NR = NCH // RW  # 31 ranges, each a contiguous [126, RW*C] DRAM block
OB = 8  # psum banks per output DMA batch
N_WARM = 20  # dummy matmuls to warm the PE clock gate

_last_results = None  # BassKernelResults of the most recent run (for test.py)


def _build_program(threshold: float):
    nc = bacc.Bacc("TRN2", target_bir_lowering=False, debug=False)

    votes_d = nc.dram_tensor(
        "votes", [NR, KP, RW * C], mybir.dt.float8e4, kind="ExternalInput"
    )
    w_d = nc.dram_tensor("w", [KP, 2], mybir.dt.float16, kind="ExternalInput")
    out_d = nc.dram_tensor("out", [2, 8, OW], mybir.dt.int8, kind="ExternalOutput")

    with tile.TileContext(nc) as tc:
        with (
            tc.tile_pool(name="w", bufs=1) as wpool,
            tc.tile_pool(name="v", bufs=1) as vpool,
            tc.tile_pool(name="o", bufs=2) as opool,
            tc.tile_pool(name="ps", bufs=7, space="PSUM") as ppool,
            tc.tile_pool(name="pd", bufs=1, space="PSUM") as dpool,
        ):
            w_sb = wpool.tile([KP, 2], mybir.dt.float16, tag="w")
            nc.scalar.dma_start(out=w_sb[:], in_=w_d[:])
            negt_sb = wpool.tile([128, 1], mybir.dt.float32, tag="negt")
            nc.vector.memset(negt_sb[:], -threshold)

            # PE warm-up: dense dummy matmuls on a zeroed tile while the
            # first input ranges are still in flight (HAM un-throttle).
            dumm = wpool.tile([KP, C], mybir.dt.float8e4, tag="dumm")
            nc.vector.memset(dumm[:], 0)
            dps = dpool.tile([128, C], mybir.dt.float32)
            for _ in range(N_WARM):
                nc.tensor.matmul(
                    dps[0:2, :C],
                    dumm[:, 0:2],
                    dumm[:, :C],
                    start=True,
                    stop=True,
                    tile_position=(0, 0),
                )

            vt = vpool.tile([KP, W], mybir.dt.float8e4, tag="v")
            for r in range(NR):
                q = nc.sync if r % 2 == 0 else nc.gpsimd
                q.dma_start(
                    out=vt[:, r * RW * C : (r + 1) * RW * C],
                    in_=votes_d[r],
                )

            ps = None
            ot = None
            for c in range(NCH):
                j = c % 4
                h, pos = divmod(c // 4, HGRP)  # output half, bank within half
                if j == 0:
                    ps = ppool.tile([128, C], mybir.dt.float32)
                nc.tensor.matmul(
                    ps[32 * j : 32 * j + 2, :C],
                    w_sb[:],
                    vt[:, c * C : (c + 1) * C],
                    start=True,
                    stop=True,
                    tile_position=(0, 32 * j),
                )

                if j == 3:
                    if pos == 0:
                        ot = opool.tile([128, OW], mybir.dt.int8)
                    osl = ot[0:98, pos * C : (pos + 1) * C]
                    if pos % 2 == 0:
                        nc.vector.tensor_scalar(
                            out=osl,
                            in0=ps[0:98, :C],
                            scalar1=threshold,
                            scalar2=None,
                            op0=mybir.AluOpType.is_gt,
                        )
                    else:
                        nc.scalar.activation(
                            out=osl,
                            in_=ps[0:98, :C],
                            func=mybir.ActivationFunctionType.Sign,
                            bias=negt_sb[0:98, :],
                            scale=1.0,
                        )
                    # batched output DMA every OB banks (and at half end)
                    endpos = pos + 1
                    if endpos % OB == 0 or endpos == HGRP:
                        p0 = (endpos - 1) // OB * OB
                        for j2 in range(4):
                            nc.scalar.dma_start(
                                out=out_d[h, 2 * j2 : 2 * j2 + 2, p0 * C : endpos * C],
                                in_=ot[32 * j2 : 32 * j2 + 2, p0 * C : endpos * C],
                            )

    nc.compile()
    return nc


def kernel(votes: np.ndarray, vote_weights: np.ndarray) -> np.ndarray:
    global _last_results
    votes = np.ascontiguousarray(votes, dtype=np.int32)
    w = np.asarray(vote_weights, dtype=np.float32)
    assert votes.shape == (N_MODELS, N_SAMPLES)

    w16 = w.astype(np.float16)
    threshold = float(w16.astype(np.float64).sum() / 2.0)
    w_sb = np.zeros((KP, 2), np.float16)
    w_sb[:N_MODELS, 0] = w16
    w_sb[N_MODELS:, 1] = w16

    # votes {0,1} -> e4m3 bytes {0x00, 0x38} ({0.0, 1.0})
    v8 = (votes.astype(np.uint8) * 0x38).astype(np.uint8)

    in_maps = []
    for core in range(N_CORES):
        sh = v8[:, core * S_CORE : (core + 1) * S_CORE]
        folded = np.zeros((KP, W), np.uint8)
        folded[:N_MODELS, :H] = sh[:, :H]
        folded[N_MODELS:, :H] = sh[:, H:]
        ranged = np.ascontiguousarray(
            folded.reshape(KP, NR, RW * C).transpose(1, 0, 2)
        )
        in_maps.append(
            {"votes": ranged.view(ml_dtypes.float8_e4m3), "w": w_sb}
        )

    nc = _build_program(threshold)
    res = bass_utils.run_bass_kernel_spmd(nc, in_maps, core_ids=list(range(N_CORES)))
    _last_results = res

    out = np.empty(N_SAMPLES, np.int32)
    for core in range(N_CORES):
        arr = np.asarray(res.results[core]["out"]).view(np.int8)
        # [2, 8, OW] -> axes (h, j, f, pos, k)
        arr = arr.reshape(2, 4, 2, HGRP, C)
        y = np.empty((2, NCH, C), np.int8)
        for h in range(2):
            for j in range(4):
                y[:, h * (NCH // 2) + j : (h + 1) * (NCH // 2) : 4, :] = arr[h, j]
        dec = (y.reshape(2, W)[:, :H] > 0).astype(np.int32)
        out[core * S_CORE : core * S_CORE + H] = dec[0]
        out[core * S_CORE + H : (core + 1) * S_CORE] = dec[1]
    return out


# revision 15
# speedup vs baseline: 1.0980x; 1.0980x over previous
"""HardBinaryVote Trainium2 kernel.

out[s] = (sum_m w[m]*votes[m,s] > sum_m w[m]/2)  as int32, votes in {0,1}.

Strategy (8 NeuronCores, sample-sharded):
  - Each core gets a [63, 250000] shard of votes, folded host-side into
    [126, 125000] (two fold-halves stacked on the partition axis), padded to
    126976 columns (248 chunks of 512), encoded as fp8 e4m3 {0.0, 1.0}
    (1 byte/vote -> plain HWDGE DMA at the ~360 GB/s HBM-per-core limit).
    Input is pre-tiled host-side into 31 contiguous 516 KB ranges
    ([31, 126, 4096]) so every range DMA is a single linear DRAM read;
    ranges alternate between the sync and gpsimd queues.
  - Weights quantized once to fp16 (exact-decision mismatch count vs the
    fp32 reference measured at 135/2M, rel_err 0.012 < 2e-2), laid out
    [126, 2] block-diagonal over the two fold-halves.
  - Single matmul pass, mixed dtype (e4m3 moving x fp16 stationary), with
    4-way PE column tiling: chunks round-robin tile_position (0, 32j), so
    4 matmuls stream concurrently (~61 ns per 512-col matmul when warm).
    A short dummy-matmul burst at kernel start warms the PE clock gate
    (HAM) while the first input ranges are still in flight.
  - Per PSUM bank (4 chunks), threshold alternates between DVE
    tensor_scalar(is_gt, T) -> {0,1} and ACT Sign(y - T) -> {-1,0,1};
    host maps >0 to 1. int8 outputs, batched sparse-partition DMAs.
"""

import sys

import numpy as np

sys.path.insert(0, "/opt/trn_rl_repo")

import ml_dtypes  # noqa: E402

from concourse import bacc, bass_utils, mybir, tile  # noqa: E402

N_MODELS = 63
N_SAMPLES = 2_000_000
N_CORES = 8
S_CORE = N_SAMPLES // N_CORES  # 250000 samples per core
H = S_CORE // 2  # 125000 real columns per core (2 samples each)
KP = 2 * N_MODELS  # 126 contraction rows

C = 512  # matmul free dim / PSUM bank
NCH = 248  # chunks per core (padded)
W = NCH * C  # 126976 padded columns
HGRP = NCH // 8  # 31 psum banks per output half (bank = 4 chunks)
OW = HGRP * C  # 15872 output columns per half per (j, fold) row

RW = 8  # chunks per input DMA range
NR = NCH // RW  # 31 ranges, each a contiguous [126, RW*C] DRAM block
OB = 8  # psum banks per output DMA batch
N_WARM = 20  # dummy matmuls to warm the PE clock gate

_last_results = None  # BassKernelResults of the most recent run (for test.py)


def _build_program(threshold: float):
    nc = bacc.Bacc("TRN2", target_bir_lowering=False, debug=False)

    votes_d = nc.dram_tensor(
        "votes", [NR, KP, RW * C], mybir.dt.float8e4, kind="ExternalInput"
    )
    w_d = nc.dram_tensor("w", [KP, 2], mybir.dt.float16, kind="ExternalInput")
    out_d = nc.dram_tensor("out", [2, 8, OW], mybir.dt.int8, kind="ExternalOutput")

    with tile.TileContext(nc) as tc:
        with (
            tc.tile_pool(name="w", bufs=1) as wpool,
            tc.tile_pool(name="v", bufs=1) as vpool,
            tc.tile_pool(name="o", bufs=2) as opool,
            tc.tile_pool(name="ps", bufs=7, space="PSUM") as ppool,
            tc.tile_pool(name="pd", bufs=1, space="PSUM") as dpool,
        ):
            w_sb = wpool.tile([KP, 2], mybir.dt.float16, tag="w")
            nc.scalar.dma_start(out=w_sb[:], in_=w_d[:])
            negt_sb = wpool.tile([128, 1], mybir.dt.float32, tag="negt")
            nc.vector.memset(negt_sb[:], -threshold)

            # PE warm-up: dense dummy matmuls on a zeroed tile while the
            # first input ranges are still in flight (HAM un-throttle).
            dumm = wpool.tile([KP, C], mybir.dt.float8e4, tag="dumm")
            nc.vector.memset(dumm[:], 0)
            dps = dpool.tile([128, C], mybir.dt.float32)
            for _ in range(N_WARM):
                nc.tensor.matmul(
                    dps[0:2, :C],
                    dumm[:, 0:2],
                    dumm[:, :C],
                    start=True,
                    stop=True,
                    tile_position=(0, 0),
                )

            vt = vpool.tile([KP, W], mybir.dt.float8e4, tag="v")
            for r in range(NR):
                nc.sync.dma_start(
                    out=vt[:, r * RW * C : (r + 1) * RW * C],
                    in_=votes_d[r],
                )

            ps = None
            ot = None
            for c in range(NCH):
                j = c % 4
                h, pos = divmod(c // 4, HGRP)  # output half, bank within half
                if j == 0:
                    # dummy drip: keeps the PE array busy across DMA jitter so
                    # the HAM clock gate never re-throttles (strict FIFO means
                    # these run while the next group's input DMA completes)
                    for _ in range(3):
                        nc.tensor.matmul(
                            dps[0:2, :256],
                            dumm[:, 0:2],
                            dumm[:, :256],
                            start=True,
                            stop=True,
                            tile_position=(0, 0),
                        )
                    ps = ppool.tile([128, C], mybir.dt.float32)
                nc.tensor.matmul(
                    ps[32 * j : 32 * j + 2, :C],
                    w_sb[:],
                    vt[:, c * C : (c + 1) * C],
                    start=True,
                    stop=True,
                    tile_position=(0, 32 * j),
                )

                if j == 3:
                    if pos == 0:
                        ot = opool.tile([128, OW], mybir.dt.int8)
                    osl = ot[0:98, pos * C : (pos + 1) * C]
                    if pos % 2 == 0:
                        nc.vector.tensor_scalar(
                            out=osl,
                            in0=ps[0:98, :C],
                            scalar1=threshold,
                            scalar2=None,
                            op0=mybir.AluOpType.is_gt,
                        )
                    else:
                        nc.scalar.activation(
                            out=osl,
                            in_=ps[0:98, :C],
                            func=mybir.ActivationFunctionType.Sign,
                            bias=negt_sb[0:98, :],
                            scale=1.0,
                        )
                    # batched output DMA every OB banks (and at half end)
                    endpos = pos + 1
                    if endpos % OB == 0 or endpos == HGRP:
                        p0 = (endpos - 1) // OB * OB
                        for j2 in range(4):
                            nc.scalar.dma_start(
                                out=out_d[h, 2 * j2 : 2 * j2 + 2, p0 * C : endpos * C],
                                in_=ot[32 * j2 : 32 * j2 + 2, p0 * C : endpos * C],
                            )

    nc.compile()
    return nc


def kernel(votes: np.ndarray, vote_weights: np.ndarray) -> np.ndarray:
    global _last_results
    votes = np.ascontiguousarray(votes, dtype=np.int32)
    w = np.asarray(vote_weights, dtype=np.float32)
    assert votes.shape == (N_MODELS, N_SAMPLES)

    w16 = w.astype(np.float16)
    threshold = float(w16.astype(np.float64).sum() / 2.0)
    w_sb = np.zeros((KP, 2), np.float16)
    w_sb[:N_MODELS, 0] = w16
    w_sb[N_MODELS:, 1] = w16

    # votes {0,1} -> e4m3 bytes {0x00, 0x38} ({0.0, 1.0})
    v8 = (votes.astype(np.uint8) * 0x38).astype(np.uint8)

    in_maps = []
    for core in range(N_CORES):
        sh = v8[:, core * S_CORE : (core + 1) * S_CORE]
        folded = np.zeros((KP, W), np.uint8)
        folded[:N_MODELS, :H] = sh[:, :H]
        folded[N_MODELS:, :H] = sh[:, H:]
        ranged = np.ascontiguousarray(
            folded.reshape(KP, NR, RW * C).transpose(1, 0, 2)
        )
        in_maps.append(
            {"votes": ranged.view(ml_dtypes.float8_e4m3), "w": w_sb}
        )

    nc = _build_program(threshold)
    res = bass_utils.run_bass_kernel_spmd(nc, in_maps, core_ids=list(range(N_CORES)))
    _last_results = res

    out = np.empty(N_SAMPLES, np.int32)
    for core in range(N_CORES):
        arr = np.asarray(res.results[core]["out"]).view(np.int8)
        # [2, 8, OW] -> axes (h, j, f, pos, k)
        arr = arr.reshape(2, 4, 2, HGRP, C)
        y = np.empty((2, NCH, C), np.int8)
        for h in range(2):
            for j in range(4):
                y[:, h * (NCH // 2) + j : (h + 1) * (NCH // 2) : 4, :] = arr[h, j]
        dec = (y.reshape(2, W)[:, :H] > 0).astype(np.int32)
        out[core * S_CORE : core * S_CORE + H] = dec[0]
        out[core * S_CORE + H : (core + 1) * S_CORE] = dec[1]
    return out


# revision 18
# speedup vs baseline: 1.2696x; 1.1564x over previous
"""HardBinaryVote Trainium2 kernel.

out[s] = (sum_m w[m]*votes[m,s] > sum_m w[m]/2)  as int32, votes in {0,1}.

Strategy (8 NeuronCores, sample-sharded):
  - Each core gets a [63, 250000] shard of votes, folded host-side into
    [126, 125000] (two fold-halves stacked on the partition axis), padded to
    126976 columns (248 chunks of 512), encoded as fp8 e4m3 {0.0, 1.0}
    (1 byte/vote -> plain HWDGE DMA at the ~360 GB/s HBM-per-core limit).
    Input is pre-tiled host-side into 31 contiguous 516 KB ranges
    ([31, 126, 4096]) so every range DMA is a single linear DRAM read;
    ranges alternate between the sync and gpsimd queues.
  - Weights quantized once to fp16 (exact-decision mismatch count vs the
    fp32 reference measured at 135/2M, rel_err 0.012 < 2e-2), laid out
    [126, 2] block-diagonal over the two fold-halves.
  - Single matmul pass, mixed dtype (e4m3 moving x fp16 stationary), with
    4-way PE column tiling: chunks round-robin tile_position (0, 32j), so
    4 matmuls stream concurrently (~61 ns per 512-col matmul when warm).
    A short dummy-matmul burst at kernel start warms the PE clock gate
    (HAM) while the first input ranges are still in flight.
  - Per PSUM bank (4 chunks), threshold alternates between DVE
    tensor_scalar(is_gt, T) -> {0,1} and ACT Sign(y - T) -> {-1,0,1};
    host maps >0 to 1. int8 outputs, batched sparse-partition DMAs.
"""

import sys

import numpy as np

sys.path.insert(0, "/opt/trn_rl_repo")

import ml_dtypes  # noqa: E402

from concourse import bacc, bass_utils, mybir, tile  # noqa: E402

N_MODELS = 63
N_SAMPLES = 2_000_000
N_CORES = 8
S_CORE = N_SAMPLES // N_CORES  # 250000 samples per core
H = S_CORE // 2  # 125000 real columns per core (2 samples each)
KP = 2 * N_MODELS  # 126 contraction rows

C = 512  # matmul free dim / PSUM bank
NCH = 248  # chunks per core (padded)
W = NCH * C  # 126976 padded columns
HGRP = NCH // 8  # 31 psum banks per output half (bank = 4 chunks)
OW = HGRP * C  # 15872 output columns per half per (j, fold) row

RW = 8  # chunks per input DMA range
NR = NCH // RW  # 31 ranges, each a contiguous [126, RW*C] DRAM block
OB = 16  # psum banks per output DMA batch
GATE = 20  # real matmuls start once this many ranges have landed
N_WARM = 6  # dense dummy matmuls at gate-open (HAM warm-up)

_last_results = None  # BassKernelResults of the most recent run (for test.py)


def _build_program(threshold: float):
    nc = bacc.Bacc("TRN2", target_bir_lowering=False, debug=False)

    votes_d = nc.dram_tensor(
        "votes", [NR, KP, RW * C], mybir.dt.float8e4, kind="ExternalInput"
    )
    w_d = nc.dram_tensor("w", [KP, 2], mybir.dt.float16, kind="ExternalInput")
    out_d = nc.dram_tensor("out", [2, 8, OW], mybir.dt.int8, kind="ExternalOutput")

    with tile.TileContext(nc) as tc:
        with (
            tc.tile_pool(name="w", bufs=1) as wpool,
            tc.tile_pool(name="v", bufs=1) as vpool,
            tc.tile_pool(name="o", bufs=2) as opool,
            tc.tile_pool(name="ps", bufs=3, space="PSUM") as ppool,
            tc.tile_pool(name="pd", bufs=1, space="PSUM") as dpool,
        ):
            w_sb = wpool.tile([KP, 2], mybir.dt.float16, tag="w")
            nc.scalar.dma_start(out=w_sb[:], in_=w_d[:])
            negt_sb = wpool.tile([128, 1], mybir.dt.float32, tag="negt")
            nc.vector.memset(negt_sb[:], -threshold)

            dumm = wpool.tile([KP, C], mybir.dt.float8e4, tag="dumm")
            nc.vector.memset(dumm[:], 0)
            dps = dpool.tile([128, C], mybir.dt.float32)

            vt = vpool.tile([KP, W], mybir.dt.float8e4, tag="v")
            for r in range(NR):
                nc.sync.dma_start(
                    out=vt[:, r * RW * C : (r + 1) * RW * C],
                    in_=votes_d[r],
                )

            # Gate drip: one throwaway matmul per landed range. The PE queue
            # is FIFO, so the real train below starts only once GATE ranges
            # are resident -- late enough that it then never data-stalls and
            # finishes together with the DMA stream.
            for r in range(GATE):
                nc.tensor.matmul(
                    dps[0:2, :256],
                    dumm[:, 0:2],
                    vt[:, r * RW * C : r * RW * C + 256],
                    start=True,
                    stop=True,
                    tile_position=(0, 0),
                )
            # dense warm-up burst at gate-open (HAM un-throttle)
            for _ in range(N_WARM):
                nc.tensor.matmul(
                    dps[0:2, :C],
                    dumm[:, 0:2],
                    dumm[:, :C],
                    start=True,
                    stop=True,
                    tile_position=(0, 0),
                )

            ps = None
            ot = None
            for c in range(NCH):
                j = c % 4
                h, pos = divmod(c // 4, HGRP)  # output half, bank within half
                b = pos % 2  # bank within the 2-bank psum tile
                if j == 0 and b == 0:
                    ps = ppool.tile([128, 2, C], mybir.dt.float32)
                nc.tensor.matmul(
                    ps[32 * j : 32 * j + 2, b, :C],
                    w_sb[:],
                    vt[:, c * C : (c + 1) * C],
                    start=True,
                    stop=True,
                    tile_position=(0, 32 * j),
                )

                if j == 3 and (b == 1 or pos == HGRP - 1):
                    pos0 = pos - b  # first bank of this psum tile
                    nb = b + 1
                    if pos0 == 0:
                        ot = opool.tile([128, OW], mybir.dt.int8)
                    osl = ot[0:98, pos0 * C : (pos0 + nb) * C]
                    if (pos0 // 2) % 2 == 0:
                        nc.vector.tensor_scalar(
                            out=osl,
                            in0=ps[0:98, :nb, :C],
                            scalar1=threshold,
                            scalar2=None,
                            op0=mybir.AluOpType.is_gt,
                        )
                    else:
                        nc.scalar.activation(
                            out=osl,
                            in_=ps[0:98, :nb, :C],
                            func=mybir.ActivationFunctionType.Sign,
                            bias=negt_sb[0:98, :],
                            scale=1.0,
                        )
                    # batched output DMA every OB banks (and at half end)
                    endpos = pos0 + nb
                    if endpos % OB == 0 or endpos == HGRP:
                        p0 = (endpos - 1) // OB * OB
                        for j2 in range(4):
                            nc.scalar.dma_start(
                                out=out_d[h, 2 * j2 : 2 * j2 + 2, p0 * C : endpos * C],
                                in_=ot[32 * j2 : 32 * j2 + 2, p0 * C : endpos * C],
                            )

    nc.compile()
    return nc


def kernel(votes: np.ndarray, vote_weights: np.ndarray) -> np.ndarray:
    global _last_results
    votes = np.ascontiguousarray(votes, dtype=np.int32)
    w = np.asarray(vote_weights, dtype=np.float32)
    assert votes.shape == (N_MODELS, N_SAMPLES)

    w16 = w.astype(np.float16)
    threshold = float(w16.astype(np.float64).sum() / 2.0)
    w_sb = np.zeros((KP, 2), np.float16)
    w_sb[:N_MODELS, 0] = w16
    w_sb[N_MODELS:, 1] = w16

    # votes {0,1} -> e4m3 bytes {0x00, 0x38} ({0.0, 1.0})
    v8 = (votes.astype(np.uint8) * 0x38).astype(np.uint8)

    in_maps = []
    for core in range(N_CORES):
        sh = v8[:, core * S_CORE : (core + 1) * S_CORE]
        folded = np.zeros((KP, W), np.uint8)
        folded[:N_MODELS, :H] = sh[:, :H]
        folded[N_MODELS:, :H] = sh[:, H:]
        ranged = np.ascontiguousarray(
            folded.reshape(KP, NR, RW * C).transpose(1, 0, 2)
        )
        in_maps.append(
            {"votes": ranged.view(ml_dtypes.float8_e4m3), "w": w_sb}
        )

    nc = _build_program(threshold)
    res = bass_utils.run_bass_kernel_spmd(nc, in_maps, core_ids=list(range(N_CORES)))
    _last_results = res

    out = np.empty(N_SAMPLES, np.int32)
    for core in range(N_CORES):
        arr = np.asarray(res.results[core]["out"]).view(np.int8)
        # [2, 8, OW] -> axes (h, j, f, pos, k)
        arr = arr.reshape(2, 4, 2, HGRP, C)
        y = np.empty((2, NCH, C), np.int8)
        for h in range(2):
            for j in range(4):
                y[:, h * (NCH // 2) + j : (h + 1) * (NCH // 2) : 4, :] = arr[h, j]
        dec = (y.reshape(2, W)[:, :H] > 0).astype(np.int32)
        out[core * S_CORE : core * S_CORE + H] = dec[0]
        out[core * S_CORE + H : (core + 1) * S_CORE] = dec[1]
    return out
